# revision 7
# baseline (speedup 1.0000x reference)
"""Trainium2 Bass kernel for DecoderAttention (B=16, T=1024, D=1024, H=16).

Data-parallel over batch (16 items / 8 cores = 2 per core), bf16 matmuls.

Pipeline structure: the next head-pair's Q/K projection is emitted in four
512-column psum chunks interleaved into the current pair's exp-bound kt
loop (the last chunk ahead of the final AV so its bias-add frees the psum
ring slot in time); Q/K destinations are split into half-tiles so logits
only wait on the half they read; softmax normalization (reciprocal -> Pool
partition_broadcast -> multiply) follows the last AV so its psum slots
free before the next pair's first AV; Q/K weight tiles prefetch one pair
ahead on the SP queue; bias DMAs ride the ACT queue; the out-projection
epilogue runs per 512-column half in its own psum tile; V tiles are
padded to 128 columns per head so the AV matmuls get the compiler's
automatic Fast Weight Load.

Host-side (free): hs is transposed to hsT [BL, D, T] and converted to
bf16; w_qkv / w_out converted to bf16. Device then runs:
  - V projection (natural [t,j] layout + fused ones-column per head so the
    AV matmul emits the softmax denominator for free)
  - per head-pair: Q/K projection into transposed [j,t] bf16 tiles, then
    per k-tile: QK^T logits (two heads row-packed on 64-row PE groups),
    exp on ScalarE (scale folded), AV accumulate; normalization:
    reciprocal (DVE) -> partition_broadcast (Pool) -> multiply (DVE).
  - out-projection from attn_outT.

ROWSPLIT=True additionally emits every 128-contraction matmul as two
64-row matmuls on alternating PE row groups (base partitions 0/64), which
the hardware can execute concurrently (the cost-model sim does not model
this and will charge ~2x for those windows).
"""

import os
import sys

import numpy as np

sys.path.insert(0, "/opt/trn_rl_repo")

import concourse.bass as bass  # noqa: E402
import concourse.mybir as mybir  # noqa: E402
import concourse.tile as tile  # noqa: E402
from concourse import bacc  # noqa: E402
from concourse.bass_utils import run_bass_kernel_spmd  # noqa: E402

F32 = mybir.dt.float32
BF16 = mybir.dt.bfloat16

B, T, D = 16, 1024, 1024
H, HD = 16, 64
N_CORES = 8
BL = B // N_CORES
P = 128
CT = D // P  # 8 contraction tiles
TT = T // P  # 8 token tiles
NQ = 512
SCALE = 1.0 / np.sqrt(HD)

ROWSPLIT = False

_last_results = None


def build_program(rowsplit=None):
    if rowsplit is None:
        rowsplit = ROWSPLIT
    nc = bacc.Bacc(
        "TRN2", target_bir_lowering=False, debug=False, num_devices=N_CORES
    )

    hsT = nc.dram_tensor("hsT", [BL, D, T], BF16, kind="ExternalInput")
    w_qkv = nc.dram_tensor("w_qkv", [D, 3 * D], BF16, kind="ExternalInput")
    b_qkv = nc.dram_tensor("b_qkv", [3 * D], F32, kind="ExternalInput")
    w_out = nc.dram_tensor("w_out", [D, D], BF16, kind="ExternalInput")
    b_out = nc.dram_tensor("b_out", [D], F32, kind="ExternalInput")
    out = nc.dram_tensor("out", [BL, T, D], F32, kind="ExternalOutput")

    Exp = mybir.ActivationFunctionType.Exp
    add = mybir.AluOpType.add
    mult = mybir.AluOpType.mult

    def acc(ps, pairs, rowsplit=rowsplit):
        """Accumulating matmul stream: pairs = [(lhsT, rhs), ...] all K=128.
        In rowsplit mode each is emitted as two 64-row matmuls alternating
        PE row groups so hardware can overlap them."""
        n = len(pairs)
        if not rowsplit:
            for i, (lt, rh) in enumerate(pairs):
                nc.tensor.matmul(ps, lt, rh, start=(i == 0), stop=(i == n - 1))
        else:
            for i, (lt, rh) in enumerate(pairs):
                for g in range(2):
                    nc.tensor.matmul(
                        ps,
                        lt[g * 64 : (g + 1) * 64, :],
                        rh[g * 64 : (g + 1) * 64, :],
                        start=(i == 0 and g == 0),
                        stop=(i == n - 1 and g == 1),
                    )

    with tile.TileContext(nc) as tc:
        with (
            tc.tile_pool(name="consts", bufs=1) as consts,
            tc.tile_pool(name="main", bufs=1) as main,
            tc.tile_pool(name="pipe", bufs=2) as pipe,
            tc.tile_pool(name="psum", bufs=1, space="PSUM") as psum,
        ):
            # ---------------- constants / weights (resident) -------------
            ones_ph = consts.tile([P, H, 1], BF16)
            nc.gpsimd.memset(ones_ph, 1.0)
            # per-partition bias for QT/KT tiles: bq[p, jt] = b_qkv[jt*128+p]
            bq = consts.tile([P, 2 * CT], F32)
            nc.scalar.dma_start(
                out=bq, in_=b_qkv.rearrange("(i p) -> p i", p=P)[:, 0 : 2 * CT]
            )
            bv_row = consts.tile([1, D], F32)
            nc.scalar.dma_start(out=bv_row, in_=b_qkv[2 * D : 3 * D][None, :])
            bout_row = consts.tile([1, D], F32)
            nc.scalar.dma_start(out=bout_row, in_=b_out[None, :])
            bcast_bv = consts.tile([P, D], F32)
            nc.gpsimd.partition_broadcast(bcast_bv, bv_row)
            bcast_bout = consts.tile([P, D], F32)
            nc.gpsimd.partition_broadcast(bcast_bout, bout_row)

            # V-projection weights [d-part, c, j] once per core
            wv_sb = []
            for c in range(CT):
                wv_t = consts.tile([P, D], BF16, name=f"wv{c}")
                wv_eng = nc.sync if c % 2 == 0 else nc.gpsimd
                wv_eng.dma_start(
                    out=wv_t, in_=w_qkv[c * P : (c + 1) * P, 2 * D : 3 * D]
                )
                wv_sb.append(wv_t)
            # out-projection weights once per core
            wout_sb = []
            for dt in range(CT):
                wo_t = consts.tile([P, D], BF16, name=f"wo{dt}")
                nc.scalar.dma_start(
                    out=wo_t, in_=w_out[dt * P : (dt + 1) * P, :]
                )
                wout_sb.append(wo_t)

            for b in range(BL):
                # ------- hsT tiles straight from HBM --------------------
                hsT_sb = []
                for c in range(CT):
                    h_t = main.tile([P, T], BF16, tag=f"hsT{c}", bufs=2,
                                    name=f"hsT{b}_{c}")
                    dma_eng = nc.sync if c % 2 == 0 else nc.gpsimd
                    dma_eng.dma_start(
                        out=h_t, in_=hsT[b, c * P : (c + 1) * P, :]
                    )
                    hsT_sb.append(h_t)

                # ------- V-projection -----------------------------------
                V = []
                for t in range(TT):
                    ps_v = psum.tile([P, D], F32, tag="p_av", bufs=2,
                                     name=f"ps_v{b}_{t}")
                    for q in range(2):
                        sl = slice(q * NQ, (q + 1) * NQ)
                        acc(
                            ps_v[:, sl],
                            [
                                (hsT_sb[c][:, t * P : (t + 1) * P],
                                 wv_sb[c][:, sl])
                                for c in range(CT)
                            ],
                        )
                    # V tiles padded to 128 columns per head (data 0-63,
                    # ones at 64 for the softmax denominator, zeros beyond):
                    # a 128-column stationary operand triggers the
                    # compiler's automatic Fast Weight Load for the AV
                    # matmuls, hiding LDWEIGHTS behind the moving stream.
                    v_t = main.tile([P, H, P], BF16, tag=f"v{t}",
                                    name=f"V{b}_{t}")
                    if b == 0:
                        nc.gpsimd.memset(v_t[:, :, HD + 1 :], 0.0)
                    nc.vector.tensor_copy(v_t[:, :, HD : HD + 1], ones_ph)
                    with nc.allow_low_precision(reason="attn V in bf16"):
                        nc.vector.tensor_tensor(
                            out=v_t[:, :, 0:HD],
                            in0=ps_v.rearrange("p (h e) -> p h e", h=H),
                            in1=bcast_bv.rearrange("p (h e) -> p h e", h=H),
                            op=add,
                        )
                    V.append(v_t)

                # ------- C-window: per head pair ------------------------
                attnT = [
                    main.tile([P, T], BF16, tag=f"attnT{g}", bufs=2,
                              name=f"attnT{b}_{g}")
                    for g in range(CT)
                ]

                def emit_norm_copy(b, hp, ps_av, i):
                    # DVE reciprocal drains the denominator row to SBUF
                    # (gpsimd has no PSUM port), Pool broadcasts it
                    h = 2 * hp + i
                    den_sb = main.tile([1, T], F32, tag=f"den{i}", bufs=2,
                                       name=f"den{b}_{h}")
                    with nc.allow_low_precision(reason="softmax denom recip"):
                        nc.vector.reciprocal(
                            den_sb, ps_av[i][HD : HD + 1, :]
                        )
                    bc_sb = pipe.tile([HD, T], F32, tag="bc_sb", bufs=2,
                                      name=f"bc{b}_{h}")
                    for q in range(2):
                        sl = slice(q * NQ, (q + 1) * NQ)
                        nc.gpsimd.partition_broadcast(
                            bc_sb[:, sl], den_sb[:, sl]
                        )
                    return bc_sb

                def emit_norm_div(b, hp, ps_av, i, bc_sb):
                    g, r0 = hp, i * HD
                    with nc.allow_low_precision(reason="attn out in bf16"):
                        nc.vector.tensor_tensor(
                            out=attnT[g][r0 : r0 + HD, :],
                            in0=ps_av[i][0:HD, :], in1=bc_sb, op=mult,
                        )

                def emit_wq_dma(hp):
                    # prefetch the pair's Q/K weight tiles one pair ahead
                    tiles = {}
                    for jt in (CT + hp, hp):
                        wq_t = pipe.tile([P, CT, P], BF16, tag="wq", bufs=4,
                                         name=f"wq{b}_{jt}")
                        nc.sync.dma_start(
                            out=wq_t,
                            in_=w_qkv.rearrange("(c p) j -> p c j", p=P)[
                                :, :, jt * P : (jt + 1) * P
                            ],
                        )
                        tiles[jt] = wq_t
                    return tiles

                def proj_chunks(hp, wq_tiles):
                    # 4 generator steps: (K,q0), (K,q1), (Q,q0), (Q,q1).
                    # Each 512-half gets its own psum tile AND its own
                    # destination SBUF tile: consumers then only wait for
                    # the half they actually read (tile-granularity dep
                    # tracking would otherwise gate the next pair's first
                    # logits on the last quarter's bias-add).
                    pair_dst = {}
                    for which, jt in (("k", CT + hp), ("q", hp)):
                        wq_t = wq_tiles[jt]
                        halves = []
                        for q in range(2):
                            sl = slice(q * NQ, (q + 1) * NQ)
                            dst = main.tile([P, NQ], BF16,
                                            tag=f"{which}t{q}", bufs=2,
                                            name=f"{which.upper()}T{b}_{hp}_{q}")
                            halves.append(dst)
                            ps_h = psum.tile([P, NQ], F32, tag="p_big",
                                             bufs=2,
                                             name=f"ps_qk{b}_{jt}_{q}")
                            acc(
                                ps_h,
                                [
                                    (wq_t[:, c, :], hsT_sb[c][:, sl])
                                    for c in range(CT)
                                ],
                            )
                            with nc.allow_low_precision(reason="qk in bf16"):
                                nc.vector.tensor_scalar_add(
                                    dst, ps_h, bq[:, jt : jt + 1]
                                )
                            yield
                        pair_dst[which] = halves
                    yield pair_dst["q"], pair_dst["k"]

                def emit_proj(hp, wq_tiles):
                    *_, dst = proj_chunks(hp, wq_tiles)
                    return dst

                wq_store = {0: emit_wq_dma(0), 1: emit_wq_dma(1)}
                QTg, KTg = emit_proj(0, wq_store[0])
                for hp in range(H // 2):

                    ps_av = [
                        psum.tile([P, T], F32, tag="p_av", bufs=2,
                                  name=f"ps_av{b}_{2 * hp + i}")
                        for i in range(2)
                    ]

                    def emit_qk_exp(kt):
                        # two heads row-packed; emit in row-group-alternating
                        # order so HW can overlap the 64-row matmuls
                        ps_l = [
                            psum.tile([P, T], F32, tag="p_big", bufs=2,
                                      name=f"ps_l{b}_{hp}_{kt}_{i}")
                            for i in range(2)
                        ]
                        kh, kcol = kt // 4, (kt % 4) * P
                        for q in range(2):
                            sl = slice(q * NQ, (q + 1) * NQ)
                            for i in range(2):
                                r0 = i * HD
                                nc.tensor.matmul(
                                    ps_l[i][:, sl],
                                    KTg[kh][r0 : r0 + HD, kcol : kcol + P],
                                    QTg[q][r0 : r0 + HD, :],
                                    start=True, stop=True,
                                )
                        expts = []
                        for i in range(2):
                            expt = pipe.tile([P, T], BF16, tag="exp", bufs=3,
                                             name=f"exp{b}_{hp}_{kt}_{i}")
                            nc.scalar.activation(expt, ps_l[i], Exp,
                                                 scale=float(SCALE))
                            expts.append(expt)
                        return expts

                    def emit_av(kt, expts):
                        # AV with explicit start/stop across the kt loop
                        for i in range(2):
                            h = 2 * hp + i
                            lt_full = V[kt][:, h, :]
                            for q in range(2):
                                sl = slice(q * NQ, (q + 1) * NQ)
                                if not rowsplit:
                                    nc.tensor.matmul(
                                        ps_av[i][:, sl], lt_full,
                                        expts[i][:, sl],
                                        start=(kt == 0), stop=(kt == TT - 1),
                                    )
                                else:
                                    for g in range(2):
                                        nc.tensor.matmul(
                                            ps_av[i][:, sl],
                                            lt_full[g * 64 : (g + 1) * 64, :],
                                            expts[i][g * 64 : (g + 1) * 64, sl],
                                            start=(kt == 0 and g == 0),
                                            stop=(kt == TT - 1 and g == 1),
                                        )

                    # next pair's projection chunks ride in this pair's
                    # ACT-bound kt loop (PE slack), one chunk every 2 kt
                    if hp < H // 2 - 1:
                        chunks = proj_chunks(hp + 1, wq_store[hp + 1])
                    else:
                        chunks = None
                    next_dst = None
                    pend = emit_qk_exp(0)
                    for kt in range(1, TT):
                        nxt = emit_qk_exp(kt)
                        emit_av(kt - 1, pend)
                        pend = nxt
                        if chunks is not None and kt % 2 == 0:
                            next(chunks)
                    # tail chunk BEFORE the last AV: its bias-add then
                    # completes during AV(7), freeing the p_big slot the
                    # next pair's second logits psum needs
                    if chunks is not None:
                        next(chunks)
                        next_dst = next(chunks)
                    emit_av(TT - 1, pend)
                    # normalization frees p_av slots for the next pair;
                    # both denominator copies first, then both divides
                    bcs = [emit_norm_copy(b, hp, ps_av, i) for i in range(2)]
                    for i in range(2):
                        emit_norm_div(b, hp, ps_av, i, bcs[i])
                    if hp < H // 2 - 1:
                        if hp < H // 2 - 2:
                            wq_store[hp + 2] = emit_wq_dma(hp + 2)
                        QTg, KTg = next_dst

                # ------- out projection ---------------------------------
                for t in range(TT):
                    o_t = pipe.tile([P, D], F32, tag="obuf", name=f"o{b}_{t}")
                    for e in range(2):
                        sl = slice(e * NQ, (e + 1) * NQ)
                        ps_oh = psum.tile([P, NQ], F32, tag="p_big", bufs=2,
                                          name=f"ps_o{b}_{t}_{e}")
                        acc(
                            ps_oh,
                            [
                                (attnT[dt][:, t * P : (t + 1) * P],
                                 wout_sb[dt][:, sl])
                                for dt in range(CT)
                            ],
                        )
                        # bias + store per half in its own psum tile so the
                        # first half's epilogue overlaps the second half's
                        # matmuls without a false tile dependency
                        nc.vector.tensor_tensor(
                            out=o_t[:, sl], in0=ps_oh,
                            in1=bcast_bout[:, sl], op=add
                        )
                        out_eng = nc.gpsimd if (2 * t + e) % 2 == 0 else nc.scalar
                        out_eng.dma_start(
                            out=out[b, t * P : (t + 1) * P, sl],
                            in_=o_t[:, sl]
                        )

    nc.compile()
    return nc


_nc_cache = None


def _prep(inputs):
    import ml_dtypes

    hs = np.ascontiguousarray(
        np.asarray(inputs["hidden_states"], dtype=np.float32)
    )
    hsT = np.ascontiguousarray(
        hs.transpose(0, 2, 1).astype(ml_dtypes.bfloat16)
    )
    w_qkv = np.ascontiguousarray(
        np.asarray(inputs["w_qkv"], dtype=np.float32).astype(ml_dtypes.bfloat16)
    )
    w_out = np.ascontiguousarray(
        np.asarray(inputs["w_out"], dtype=np.float32).astype(ml_dtypes.bfloat16)
    )
    b_qkv = np.ascontiguousarray(np.asarray(inputs["b_qkv"], dtype=np.float32))
    b_out = np.ascontiguousarray(np.asarray(inputs["b_out"], dtype=np.float32))
    return hsT, w_qkv, b_qkv, w_out, b_out


def make_in_maps(inputs):
    hsT, w_qkv, b_qkv, w_out, b_out = _prep(inputs)
    return [
        {
            "hsT": hsT[c * BL : (c + 1) * BL],
            "w_qkv": w_qkv,
            "b_qkv": b_qkv,
            "w_out": w_out,
            "b_out": b_out,
        }
        for c in range(N_CORES)
    ]


def kernel(**inputs) -> np.ndarray:
    global _nc_cache, _last_results
    if _nc_cache is None:
        _nc_cache = build_program()
    nc = _nc_cache

    in_maps = make_in_maps(inputs)
    try:
        res = run_bass_kernel_spmd(
            nc,
            in_maps,
            list(range(N_CORES)),
            trace=bool(os.environ.get("BASS_TRACE")),
        )
    except ModuleNotFoundError:
        prev = os.environ.get("BASS_NEVER_TRACE")
        os.environ["BASS_NEVER_TRACE"] = "1"
        try:
            res = run_bass_kernel_spmd(nc, in_maps, list(range(N_CORES)))
        finally:
            if prev is None:
                os.environ.pop("BASS_NEVER_TRACE", None)
            else:
                os.environ["BASS_NEVER_TRACE"] = prev
    _last_results = res
    return np.concatenate(
        [res.results[c]["out"] for c in range(N_CORES)], axis=0
    )


# revision 8
# speedup vs baseline: 1.1964x; 1.1964x over previous
"""Trainium2 Bass kernel for DecoderAttention (B=16, T=1024, D=1024, H=16).

v2: data-parallel over batch (2 items/core), bf16 matmul pipeline.

Host-side (free): hs is transposed to hsT [BL, D, T] and converted to
bf16; w_qkv / w_out converted to bf16. Device then runs:
  - V projection (natural [t,j] layout + fused ones-column per head so the
    AV matmul emits the softmax denominator for free)
  - per head-pair: Q/K projection into transposed [j,t] bf16 tiles, then
    per k-tile: QK^T logits (two heads row-packed on 64-row PE groups),
    exp on ScalarE (scale folded), AV accumulate; normalization:
    reciprocal (DVE) -> partition_broadcast (Pool) -> multiply (DVE).
  - out-projection from attn_outT.

ROWSPLIT=True additionally emits every 128-contraction matmul as two
64-row matmuls on alternating PE row groups (base partitions 0/64), which
the hardware can execute concurrently (the cost-model sim does not model
this and will charge ~2x for those windows).
"""

import os
import sys

import numpy as np

sys.path.insert(0, "/opt/trn_rl_repo")

import concourse.bass as bass  # noqa: E402
import concourse.mybir as mybir  # noqa: E402
import concourse.tile as tile  # noqa: E402
from concourse import bacc  # noqa: E402
from concourse.bass_utils import run_bass_kernel_spmd  # noqa: E402

F32 = mybir.dt.float32
BF16 = mybir.dt.bfloat16

B, T, D = 16, 1024, 1024
H, HD = 16, 64
N_CORES = 8
BL = B // N_CORES
P = 128
CT = D // P  # 8 contraction tiles
TT = T // P  # 8 token tiles
NQ = 512
SCALE = 1.0 / np.sqrt(HD)

ROWSPLIT = False

_last_results = None


def build_program(rowsplit=None):
    if rowsplit is None:
        rowsplit = ROWSPLIT
    nc = bacc.Bacc(
        "TRN2", target_bir_lowering=False, debug=False, num_devices=N_CORES
    )

    hsT = nc.dram_tensor("hsT", [BL, D, T], BF16, kind="ExternalInput")
    w_qkv = nc.dram_tensor("w_qkv", [D, 3 * D], BF16, kind="ExternalInput")
    b_qkv = nc.dram_tensor("b_qkv", [3 * D], F32, kind="ExternalInput")
    w_out = nc.dram_tensor("w_out", [D, D], BF16, kind="ExternalInput")
    b_out = nc.dram_tensor("b_out", [D], F32, kind="ExternalInput")
    out = nc.dram_tensor("out", [BL, T, D], F32, kind="ExternalOutput")

    Exp = mybir.ActivationFunctionType.Exp
    add = mybir.AluOpType.add
    mult = mybir.AluOpType.mult

    def acc(ps, pairs, rowsplit=rowsplit):
        """Accumulating matmul stream: pairs = [(lhsT, rhs), ...] all K=128.
        In rowsplit mode each is emitted as two 64-row matmuls alternating
        PE row groups so hardware can overlap them."""
        n = len(pairs)
        if not rowsplit:
            for i, (lt, rh) in enumerate(pairs):
                nc.tensor.matmul(ps, lt, rh, start=(i == 0), stop=(i == n - 1))
        else:
            for i, (lt, rh) in enumerate(pairs):
                for g in range(2):
                    nc.tensor.matmul(
                        ps,
                        lt[g * 64 : (g + 1) * 64, :],
                        rh[g * 64 : (g + 1) * 64, :],
                        start=(i == 0 and g == 0),
                        stop=(i == n - 1 and g == 1),
                    )

    with tile.TileContext(nc) as tc:
        with (
            tc.tile_pool(name="consts", bufs=1) as consts,
            tc.tile_pool(name="main", bufs=1) as main,
            tc.tile_pool(name="pipe", bufs=2) as pipe,
            tc.tile_pool(name="psum", bufs=1, space="PSUM") as psum,
        ):
            # ---------------- constants / weights (resident) -------------
            ones_ph = consts.tile([P, H, 1], BF16)
            nc.gpsimd.memset(ones_ph, 1.0)
            # per-partition bias for QT/KT tiles: bq[p, jt] = b_qkv[jt*128+p]
            bq = consts.tile([P, 2 * CT], F32)
            nc.scalar.dma_start(
                out=bq, in_=b_qkv.rearrange("(i p) -> p i", p=P)[:, 0 : 2 * CT]
            )
            bv_row = consts.tile([1, D], F32)
            nc.scalar.dma_start(out=bv_row, in_=b_qkv[2 * D : 3 * D][None, :])
            bout_row = consts.tile([1, D], F32)
            nc.scalar.dma_start(out=bout_row, in_=b_out[None, :])
            bcast_bv = consts.tile([P, D], F32)
            nc.gpsimd.partition_broadcast(bcast_bv, bv_row)
            bcast_bout = consts.tile([P, D], F32)
            nc.gpsimd.partition_broadcast(bcast_bout, bout_row)

            # V-projection weights [d-part, c, j] once per core
            wv_sb = []
            for c in range(CT):
                wv_t = consts.tile([P, D], BF16, name=f"wv{c}")
                wv_eng = nc.sync if c % 2 == 0 else nc.gpsimd
                wv_eng.dma_start(
                    out=wv_t, in_=w_qkv[c * P : (c + 1) * P, 2 * D : 3 * D]
                )
                wv_sb.append(wv_t)
            # out-projection weights once per core
            wout_sb = []
            for dt in range(CT):
                wo_t = consts.tile([P, D], BF16, name=f"wo{dt}")
                nc.scalar.dma_start(
                    out=wo_t, in_=w_out[dt * P : (dt + 1) * P, :]
                )
                wout_sb.append(wo_t)

            for b in range(BL):
                # ------- hsT tiles straight from HBM --------------------
                hsT_sb = []
                for c in range(CT):
                    h_t = main.tile([P, T], BF16, tag=f"hsT{c}", bufs=2,
                                    name=f"hsT{b}_{c}")
                    dma_eng = nc.sync if c % 2 == 0 else nc.gpsimd
                    dma_eng.dma_start(
                        out=h_t, in_=hsT[b, c * P : (c + 1) * P, :]
                    )
                    hsT_sb.append(h_t)

                # ------- V-projection -----------------------------------
                V = []
                for t in range(TT):
                    ps_v = psum.tile([P, D], F32, tag="p_av", bufs=2,
                                     name=f"ps_v{b}_{t}")
                    for q in range(2):
                        sl = slice(q * NQ, (q + 1) * NQ)
                        acc(
                            ps_v[:, sl],
                            [
                                (hsT_sb[c][:, t * P : (t + 1) * P],
                                 wv_sb[c][:, sl])
                                for c in range(CT)
                            ],
                        )
                    # V tiles padded to 128 columns per head (data 0-63,
                    # ones at 64 for the softmax denominator, zeros beyond):
                    # a 128-column stationary operand triggers the
                    # compiler's automatic Fast Weight Load for the AV
                    # matmuls, hiding LDWEIGHTS behind the moving stream.
                    v_t = main.tile([P, H, P], BF16, tag=f"v{t}",
                                    name=f"V{b}_{t}")
                    if b == 0:
                        nc.gpsimd.memset(v_t[:, :, HD + 1 :], 0.0)
                    nc.vector.tensor_copy(v_t[:, :, HD : HD + 1], ones_ph)
                    with nc.allow_low_precision(reason="attn V in bf16"):
                        nc.vector.tensor_tensor(
                            out=v_t[:, :, 0:HD],
                            in0=ps_v.rearrange("p (h e) -> p h e", h=H),
                            in1=bcast_bv.rearrange("p (h e) -> p h e", h=H),
                            op=add,
                        )
                    V.append(v_t)

                # ------- C-window: per head pair ------------------------
                attnT = [
                    main.tile([P, T], BF16, tag=f"attnT{g}", bufs=2,
                              name=f"attnT{b}_{g}")
                    for g in range(CT)
                ]

                def emit_norm_copy(b, hp, ps_av, i):
                    # DVE reciprocal drains the denominator row to SBUF
                    # (gpsimd has no PSUM port), Pool broadcasts it
                    h = 2 * hp + i
                    den_sb = main.tile([1, T], F32, tag=f"den{i}", bufs=2,
                                       name=f"den{b}_{h}")
                    with nc.allow_low_precision(reason="softmax denom recip"):
                        nc.vector.reciprocal(
                            den_sb, ps_av[i][HD : HD + 1, :]
                        )
                    bc_sb = pipe.tile([HD, T], F32, tag="bc_sb", bufs=2,
                                      name=f"bc{b}_{h}")
                    for q in range(2):
                        sl = slice(q * NQ, (q + 1) * NQ)
                        nc.gpsimd.partition_broadcast(
                            bc_sb[:, sl], den_sb[:, sl]
                        )
                    return bc_sb

                def emit_norm_div(b, hp, ps_av, i, bc_sb):
                    g, r0 = hp, i * HD
                    with nc.allow_low_precision(reason="attn out in bf16"):
                        nc.vector.tensor_tensor(
                            out=attnT[g][r0 : r0 + HD, :],
                            in0=ps_av[i][0:HD, :], in1=bc_sb, op=mult,
                        )

                def emit_wq_dma(hp):
                    # prefetch the pair's Q/K weight tiles one pair ahead
                    tiles = {}
                    for jt in (CT + hp, hp):
                        wq_t = pipe.tile([P, CT, P], BF16, tag="wq", bufs=4,
                                         name=f"wq{b}_{jt}")
                        nc.sync.dma_start(
                            out=wq_t,
                            in_=w_qkv.rearrange("(c p) j -> p c j", p=P)[
                                :, :, jt * P : (jt + 1) * P
                            ],
                        )
                        tiles[jt] = wq_t
                    return tiles

                def proj_chunks(hp, wq_tiles):
                    # 4 generator steps: (K,q0), (K,q1), (Q,q0), (Q,q1).
                    # Each 512-half gets its own psum tile AND its own
                    # destination SBUF tile: consumers then only wait for
                    # the half they actually read (tile-granularity dep
                    # tracking would otherwise gate the next pair's first
                    # logits on the last quarter's bias-add).
                    pair_dst = {}
                    for which, jt in (("k", CT + hp), ("q", hp)):
                        wq_t = wq_tiles[jt]
                        halves = []
                        for q in range(2):
                            sl = slice(q * NQ, (q + 1) * NQ)
                            dst = main.tile([P, NQ], BF16,
                                            tag=f"{which}t{q}", bufs=2,
                                            name=f"{which.upper()}T{b}_{hp}_{q}")
                            halves.append(dst)
                            ps_h = psum.tile([P, NQ], F32, tag="p_big",
                                             bufs=2,
                                             name=f"ps_qk{b}_{jt}_{q}")
                            acc(
                                ps_h,
                                [
                                    (wq_t[:, c, :], hsT_sb[c][:, sl])
                                    for c in range(CT)
                                ],
                            )
                            with nc.allow_low_precision(reason="qk in bf16"):
                                nc.vector.tensor_scalar_add(
                                    dst, ps_h, bq[:, jt : jt + 1]
                                )
                            yield
                        pair_dst[which] = halves
                    yield pair_dst["q"], pair_dst["k"]

                def emit_proj(hp, wq_tiles):
                    *_, dst = proj_chunks(hp, wq_tiles)
                    return dst

                wq_store = {0: emit_wq_dma(0), 1: emit_wq_dma(1)}
                QTg, KTg = emit_proj(0, wq_store[0])
                for hp in range(H // 2):

                    ps_av = [
                        psum.tile([P, T], F32, tag="p_av", bufs=2,
                                  name=f"ps_av{b}_{2 * hp + i}")
                        for i in range(2)
                    ]

                    def emit_qk_exp(kt):
                        # two heads row-packed; emit in row-group-alternating
                        # order so HW can overlap the 64-row matmuls
                        ps_l = [
                            psum.tile([P, T], F32, tag="p_big", bufs=2,
                                      name=f"ps_l{b}_{hp}_{kt}_{i}")
                            for i in range(2)
                        ]
                        kh, kcol = kt // 4, (kt % 4) * P
                        for q in range(2):
                            sl = slice(q * NQ, (q + 1) * NQ)
                            for i in range(2):
                                r0 = i * HD
                                nc.tensor.matmul(
                                    ps_l[i][:, sl],
                                    KTg[kh][r0 : r0 + HD, kcol : kcol + P],
                                    QTg[q][r0 : r0 + HD, :],
                                    start=True, stop=True,
                                )
                        expts = []
                        for i in range(2):
                            expt = pipe.tile([P, T], BF16, tag="exp", bufs=4,
                                             name=f"exp{b}_{hp}_{kt}_{i}")
                            nc.scalar.activation(expt, ps_l[i], Exp,
                                                 scale=float(SCALE))
                            expts.append(expt)
                        return expts

                    def emit_av(kt, expts):
                        # AV with explicit start/stop across the kt loop
                        for i in range(2):
                            h = 2 * hp + i
                            lt_full = V[kt][:, h, :]
                            for q in range(2):
                                sl = slice(q * NQ, (q + 1) * NQ)
                                if not rowsplit:
                                    nc.tensor.matmul(
                                        ps_av[i][:, sl], lt_full,
                                        expts[i][:, sl],
                                        start=(kt == 0), stop=(kt == TT - 1),
                                    )
                                else:
                                    for g in range(2):
                                        nc.tensor.matmul(
                                            ps_av[i][:, sl],
                                            lt_full[g * 64 : (g + 1) * 64, :],
                                            expts[i][g * 64 : (g + 1) * 64, sl],
                                            start=(kt == 0 and g == 0),
                                            stop=(kt == TT - 1 and g == 1),
                                        )

                    # next pair's projection chunks ride in this pair's
                    # ACT-bound kt loop (PE slack), one chunk every 2 kt
                    if hp < H // 2 - 1:
                        chunks = proj_chunks(hp + 1, wq_store[hp + 1])
                    else:
                        chunks = None
                    next_dst = None
                    pend = emit_qk_exp(0)
                    for kt in range(1, TT):
                        nxt = emit_qk_exp(kt)
                        emit_av(kt - 1, pend)
                        pend = nxt
                        if chunks is not None and kt % 2 == 0:
                            next(chunks)
                    # tail chunk BEFORE the last AV: its bias-add then
                    # completes during AV(7), freeing the p_big slot the
                    # next pair's second logits psum needs
                    if chunks is not None:
                        next(chunks)
                        next_dst = next(chunks)
                    emit_av(TT - 1, pend)
                    # normalization frees p_av slots for the next pair;
                    # both denominator copies first, then both divides
                    bcs = [emit_norm_copy(b, hp, ps_av, i) for i in range(2)]
                    for i in range(2):
                        emit_norm_div(b, hp, ps_av, i, bcs[i])
                    if hp < H // 2 - 1:
                        if hp < H // 2 - 2:
                            wq_store[hp + 2] = emit_wq_dma(hp + 2)
                        QTg, KTg = next_dst

                # ------- out projection ---------------------------------
                for t in range(TT):
                    o_t = pipe.tile([P, D], F32, tag="obuf", name=f"o{b}_{t}")
                    for e in range(2):
                        sl = slice(e * NQ, (e + 1) * NQ)
                        ps_oh = psum.tile([P, NQ], F32, tag="p_big", bufs=2,
                                          name=f"ps_o{b}_{t}_{e}")
                        acc(
                            ps_oh,
                            [
                                (attnT[dt][:, t * P : (t + 1) * P],
                                 wout_sb[dt][:, sl])
                                for dt in range(CT)
                            ],
                        )
                        # bias + store per half in its own psum tile so the
                        # first half's epilogue overlaps the second half's
                        # matmuls without a false tile dependency
                        nc.vector.tensor_tensor(
                            out=o_t[:, sl], in0=ps_oh,
                            in1=bcast_bout[:, sl], op=add
                        )
                        out_eng = nc.gpsimd if (2 * t + e) % 2 == 0 else nc.scalar
                        out_eng.dma_start(
                            out=out[b, t * P : (t + 1) * P, sl],
                            in_=o_t[:, sl]
                        )

    nc.compile()
    return nc


_nc_cache = None


def _prep(inputs):
    import ml_dtypes

    hs = np.ascontiguousarray(
        np.asarray(inputs["hidden_states"], dtype=np.float32)
    )
    hsT = np.ascontiguousarray(
        hs.transpose(0, 2, 1).astype(ml_dtypes.bfloat16)
    )
    w_qkv = np.ascontiguousarray(
        np.asarray(inputs["w_qkv"], dtype=np.float32).astype(ml_dtypes.bfloat16)
    )
    w_out = np.ascontiguousarray(
        np.asarray(inputs["w_out"], dtype=np.float32).astype(ml_dtypes.bfloat16)
    )
    b_qkv = np.ascontiguousarray(np.asarray(inputs["b_qkv"], dtype=np.float32))
    b_out = np.ascontiguousarray(np.asarray(inputs["b_out"], dtype=np.float32))
    return hsT, w_qkv, b_qkv, w_out, b_out


def make_in_maps(inputs):
    hsT, w_qkv, b_qkv, w_out, b_out = _prep(inputs)
    return [
        {
            "hsT": hsT[c * BL : (c + 1) * BL],
            "w_qkv": w_qkv,
            "b_qkv": b_qkv,
            "w_out": w_out,
            "b_out": b_out,
        }
        for c in range(N_CORES)
    ]


def kernel(**inputs) -> np.ndarray:
    global _nc_cache, _last_results
    if _nc_cache is None:
        _nc_cache = build_program()
    nc = _nc_cache

    in_maps = make_in_maps(inputs)
    try:
        res = run_bass_kernel_spmd(
            nc,
            in_maps,
            list(range(N_CORES)),
            trace=bool(os.environ.get("BASS_TRACE")),
        )
    except ModuleNotFoundError:
        prev = os.environ.get("BASS_NEVER_TRACE")
        os.environ["BASS_NEVER_TRACE"] = "1"
        try:
            res = run_bass_kernel_spmd(nc, in_maps, list(range(N_CORES)))
        finally:
            if prev is None:
                os.environ.pop("BASS_NEVER_TRACE", None)
            else:
                os.environ["BASS_NEVER_TRACE"] = prev
    _last_results = res
    return np.concatenate(
        [res.results[c]["out"] for c in range(N_CORES)], axis=0
    )
